# revision 1
# baseline (speedup 1.0000x reference)
"""BigramLM embedding lookup as a distributed DMA row-gather.

Z[b,s,:] = W[inputs[b,s],:] -- the one-hot matmul in the reference is just a
row gather from a 256 MB table. Strategy: pure data parallelism over the
8*512=4096 tokens; each of the 8 cores owns 512 tokens and gathers their rows
from its full local copy of W with SWDGE dma_gather (HBM->SBUF), while HWDGE
stores finished chunks to the output (SBUF->HBM), double-buffered.

Token->slot mapping: gather chunk k, slot j holds token k*CHUNK+j, which lands
in SBUF partition j; the store then writes partition p to output row k*CHUNK+p,
so the output comes back in natural token order with no device-side reorder.
"""

import numpy as np

import concourse.bacc as bacc
import concourse.mybir as mybir
from concourse.bass_utils import run_bass_kernel_spmd
from concourse.library_config import mlp

VOCAB = 8192
EMB = 8192
BATCH, SEQ = 8, 512
N_CORES = 8
TOK = BATCH * SEQ // N_CORES  # 512 tokens per core
CHUNK = 128                   # tokens per dma_gather
K = TOK // CHUNK              # chunks per core
IDX_COLS = CHUNK // 16        # int16 idx columns per chunk (wrapped in 16 parts)
N_BUF = 2

_cache: dict = {}

# Results object of the most recent run (test.py reads exec_time_ns off it).
LAST_RESULTS = None


def _build():
    nc = bacc.Bacc("TRN2")
    w = nc.dram_tensor("w", [VOCAB, EMB], mybir.dt.float32, kind="ExternalInput")
    idx = nc.dram_tensor(
        "idx", [128, K * IDX_COLS], mybir.dt.int16, kind="ExternalInput"
    )
    out = nc.dram_tensor("out", [TOK, EMB], mybir.dt.float32, kind="ExternalOutput")
    with (
        nc.Block() as block,
        nc.sbuf_tensor("idx_sb", [128, K * IDX_COLS], mybir.dt.int16) as idx_sb,
        nc.sbuf_tensor("buf0", [128, 1, EMB], mybir.dt.float32) as buf0,
        nc.sbuf_tensor("buf1", [128, 1, EMB], mybir.dt.float32) as buf1,
        nc.semaphore("io") as io,
        nc.semaphore("gsem") as gsem,
        nc.semaphore("ssem") as ssem,
    ):
        bufs = [buf0, buf1]

        @block.gpsimd
        def _(gp):
            gp.load_library(mlp)
            gp.dma_start(idx_sb[:], idx[:]).then_inc(io, 16)
            gp.wait_ge(io, 16)
            for k in range(K):
                if k >= N_BUF:
                    # buffer reuse: store k-N_BUF must have drained
                    gp.wait_ge(ssem, 16 * (k - N_BUF + 1))
                gp.dma_gather(
                    bufs[k % N_BUF][:],
                    w[:],
                    idx_sb[:, k * IDX_COLS : (k + 1) * IDX_COLS],
                    CHUNK,
                    CHUNK,
                    EMB,
                ).then_inc(gsem, 16)

        @block.sync
        def _(sy):
            for k in range(K):
                sy.wait_ge(gsem, 16 * (k + 1))
                sy.dma_start(
                    out[k * CHUNK : (k + 1) * CHUNK, :],
                    bufs[k % N_BUF][:, 0:1, :],
                ).then_inc(ssem, 16)
            sy.wait_ge(ssem, 16 * K)

    nc.compile()
    return nc


def _pack_idx(tok_idx: np.ndarray) -> np.ndarray:
    """[TOK] token indices -> [128, K*IDX_COLS] int16 in dma_gather's wrapped
    layout: chunk k slot j (= j%16 partition-row, j//16 column) holds
    tok_idx[k*CHUNK + j], replicated across the 8 groups of 16 partitions."""
    cols = []
    for k in range(K):
        chunk = tok_idx[k * CHUNK : (k + 1) * CHUNK].reshape(IDX_COLS, 16).T
        cols.append(np.tile(chunk, (8, 1)))  # [128, IDX_COLS]
    return np.ascontiguousarray(np.concatenate(cols, axis=1), dtype=np.int16)


def kernel(inputs, W):
    global LAST_RESULTS
    inputs = np.asarray(inputs)
    W = np.ascontiguousarray(np.asarray(W, dtype=np.float32))
    flat = inputs.reshape(-1).astype(np.int64)
    assert flat.shape == (N_CORES * TOK,)
    assert flat.min() >= 0 and flat.max() < VOCAB

    nc = _cache.get("nc")
    if nc is None:
        nc = _cache["nc"] = _build()

    in_maps = [
        {"w": W, "idx": _pack_idx(flat[c * TOK : (c + 1) * TOK])}
        for c in range(N_CORES)
    ]
    res = run_bass_kernel_spmd(nc, in_maps, core_ids=list(range(N_CORES)))
    LAST_RESULTS = res
    outs = [res.results[c]["out"] for c in range(N_CORES)]
    return np.concatenate(outs, axis=0).reshape(BATCH, SEQ, EMB)


# revision 4
# speedup vs baseline: 1.0708x; 1.0708x over previous
"""BigramLM embedding lookup as a distributed DMA row-gather.

Z[b,s,:] = W[inputs[b,s],:] -- the one-hot matmul in the reference is just a
row gather from a 256 MB table. Strategy: pure data parallelism over the
8*512=4096 tokens; each of the 8 cores owns 512 tokens and gathers their rows
from its full local copy of W with SWDGE indirect DMA (HBM->SBUF), while
HWDGE stores finished chunks to the output (SBUF->HBM).

Chunk k slot p holds token k*CHUNK+p in SBUF partition p; the store writes
partition p to output row k*CHUNK+p, so the output comes back in natural
token order with no device-side reorder. One SBUF buffer per chunk -- no
buffer-reuse dependencies, all gathers are queued back to back.
"""

from contextlib import ExitStack

import numpy as np

import concourse.bacc as bacc
import concourse.bass as bass
import concourse.mybir as mybir
from concourse.bass_utils import run_bass_kernel_spmd

VOCAB = 8192
EMB = 8192
BATCH, SEQ = 8, 512
N_CORES = 8
TOK = BATCH * SEQ // N_CORES  # 512 tokens per core
CHUNK = 128                   # tokens per gather chunk (= SBUF partitions)
K = TOK // CHUNK              # chunks per core

_cache: dict = {}

# Results object of the most recent run (test.py reads exec_time_ns off it).
LAST_RESULTS = None


def _build():
    nc = bacc.Bacc("TRN2")
    w = nc.dram_tensor("w", [VOCAB, EMB], mybir.dt.float32, kind="ExternalInput")
    idx = nc.dram_tensor("idx", [CHUNK, K], mybir.dt.int32, kind="ExternalInput")
    out = nc.dram_tensor("out", [TOK, EMB], mybir.dt.float32, kind="ExternalOutput")
    with (
        nc.Block() as block,
        ExitStack() as stack,
        nc.semaphore("io") as io,
        nc.semaphore("gsem") as gsem,
        nc.semaphore("ssem") as ssem,
    ):
        idx_sb = stack.enter_context(
            nc.sbuf_tensor("idx_sb", [CHUNK, K], mybir.dt.int32)
        )
        bufs = [
            stack.enter_context(
                nc.sbuf_tensor(f"buf{k}", [CHUNK, EMB], mybir.dt.float32)
            )
            for k in range(K)
        ]

        @block.gpsimd
        def _(gp):
            gp.wait_ge(io, 16)
            for k in range(K):
                gp.indirect_dma_start(
                    out=bufs[k][:],
                    out_offset=None,
                    in_=w[:],
                    in_offset=bass.IndirectOffsetOnAxis(
                        ap=idx_sb[:, k : k + 1], axis=0
                    ),
                ).then_inc(gsem, 16)

        @block.sync
        def _(sy):
            sy.dma_start(idx_sb[:], idx[:]).then_inc(io, 16)
            for k in range(K):
                sy.wait_ge(gsem, 16 * (k + 1))
                sy.dma_start(
                    out[k * CHUNK : (k + 1) * CHUNK, :],
                    bufs[k][:],
                    single_packet=True,
                ).then_inc(ssem, 16)
            sy.wait_ge(ssem, 16 * K)

    nc.compile()
    return nc


def kernel(inputs, W):
    global LAST_RESULTS
    inputs = np.asarray(inputs)
    W = np.ascontiguousarray(np.asarray(W, dtype=np.float32))
    flat = inputs.reshape(-1).astype(np.int64)
    assert flat.shape == (N_CORES * TOK,)
    assert flat.min() >= 0 and flat.max() < VOCAB

    nc = _cache.get("nc")
    if nc is None:
        nc = _cache["nc"] = _build()

    in_maps = []
    for c in range(N_CORES):
        tok = flat[c * TOK : (c + 1) * TOK]
        # chunk k slot p = token k*CHUNK+p -> idx_sb[p, k]
        idx2d = np.ascontiguousarray(
            tok.reshape(K, CHUNK).T.astype(np.int32)
        )
        in_maps.append({"w": W, "idx": idx2d})
    res = run_bass_kernel_spmd(nc, in_maps, core_ids=list(range(N_CORES)))
    LAST_RESULTS = res
    outs = [res.results[c]["out"] for c in range(N_CORES)]
    return np.concatenate(outs, axis=0).reshape(BATCH, SEQ, EMB)
